# revision 1
# baseline (speedup 1.0000x reference)
"""Causal multi-head attention block (qkv proj + partial RoPE + causal attn +
out proj) for Trainium2, distributed over 8 NeuronCores.

Sharding: core i handles batch b = i//2 and head-group g = i%2 (6 of 12 heads).
Each core computes a partial output projection (contraction over its 6 heads'
384 channels); the host sums the two head-group partials per batch.

Design notes (evolved v2 336us -> 254us through NTFF trace analysis):
  - All matmuls bf16 (1 cyc/col; fp32r ran ~1.25 cyc/col and its power draw
    tripped the HAM 50% clock throttle). qk projection merged from 7 M-tiles
    to 6 (rope r1/r2 rows packed with pass rows into full 128-row tiles).
  - Phases interleaved: attn(jq=jt) -> proj(jt+1) -> outproj(jq=jt); the
    out-proj normalization (rowsum gather -> DVE reciprocal on [6,TQ] whose
    cost scales with FREE size -> e6 broadcast matmuls) hides under the
    next tile's projection matmuls.
  - Attention inner loop software-pipelined with lag 2: scores issue two
    k-tiles ahead of attn@v so the PE rarely waits on ACT exp (ACT costs
    (N+352)/1.2GHz per instruction and paces the attention phase).
  - proj runs its pass M-tiles (flx psum pool) BEFORE the rope tiles: the
    rope/o-accumulator pools otherwise WAR-chain each tile's projection to
    the previous attention's last PSUM eviction on the lagging ACT queue.
  - DMA discipline (queues are shape-hashed, in-order, and head-of-line
    block behind semaphore-gated entries; each dma_start costs ~600ns of
    issuing sequencer): all x tiles + weights load up front on the ACT
    ring; the SP ring carries only the qk_sb scatters; rowsum gathers and
    out stores ride the ACT ring; batched single-DMA forms are used for
    x, weights, rowsums (free->partition flatten) and the output store.
  - Rope scatter: one DMA per tensor-head; src [8, 2, TQ] free dims flatten
    into 16 dst partitions, interleaving (r1_j, r2_j) within the head's
    rope dims (a consistent q/k dim permutation leaves scores unchanged).
"""

import numpy as np

B, T, C = 4, 2048, 768
NH, HD, RD = 12, 64, 16
NHL = NH // 2          # heads per core (local)
NPAIR = NHL // 2       # head pairs per core
CL = NHL * HD          # local channels (384)
TQ = 512               # q tile
NTQ = T // TQ
NKT = T // 128         # k tiles of 128

_cache = {}


def _build(debug=False):
    import concourse.bacc as bacc
    import concourse.mybir as mybir
    import concourse.tile as tile

    F32R = mybir.dt.float32r
    F32 = mybir.dt.float32
    BF16 = mybir.dt.bfloat16
    AF = mybir.ActivationFunctionType
    MUL = mybir.AluOpType.mult
    SUB = mybir.AluOpType.subtract
    ADD = mybir.AluOpType.add

    nc = bacc.Bacc(trn_type="TRN2", name="attn8v3")

    xt = nc.dram_tensor("xt", [C, T], BF16, kind="ExternalInput")
    wqkt = nc.dram_tensor("wqkt", [C, 2 * CL], BF16, kind="ExternalInput")
    wvt = nc.dram_tensor("wvt", [C, CL], BF16, kind="ExternalInput")
    wot = nc.dram_tensor("wot", [CL, C], BF16, kind="ExternalInput")
    cosb = nc.dram_tensor("cosb", [96, T], BF16, kind="ExternalInput")
    sinb = nc.dram_tensor("sinb", [96, T], BF16, kind="ExternalInput")
    tri2 = nc.dram_tensor("tri2", [128, 2 * 128], BF16, kind="ExternalInput")
    e6 = nc.dram_tensor("e6", [6, NPAIR * 128], F32R, kind="ExternalInput")
    out = nc.dram_tensor("out", [C, T], F32, kind="ExternalOutput")

    # qk-projection M-tiles (wqkt column order, host-built):
    #   tile 0 [128] : r1 rows [96] = (q h0..h5 | k h0..h5) x dims 0:8
    #                  + r2a rows [32] = tensor-heads 0..3 x dims 8:16
    #   tile 1 [128] : r2b rows [64] = tensor-heads 4..11 x dims 8:16
    #                  + pass rows 0:64
    #   tiles 2..5   : pass rows 64:576
    # pass row order: for blk 0..5 (q pairs then k pairs):
    #   h_even dims 16:64 (48 rows), h_odd dims 16:64 (48 rows)
    A_ORDER = [0, 2, 4, 1, 3, 5, 6, 8, 10, 7, 9, 11]

    BLK_ORDER = [0, 3, 1, 4, 2, 5]

    def pass_dest(row):
        g, r = divmod(row, 96)
        part = 64 * (r // 48) + 16 + (r % 48)
        return BLK_ORDER[g], part

    with tile.TileContext(nc) as tc:
        with (
            tc.tile_pool(name="persist", bufs=1) as pp,
            tc.tile_pool(name="weights", bufs=1) as wp,
            tc.tile_pool(name="xload", bufs=4) as xlp,
            tc.tile_pool(name="pstage", bufs=2) as psg,
            tc.tile_pool(name="ropet", bufs=1) as rtp,
            tc.tile_pool(name="rots", bufs=2) as rop,
            tc.tile_pool(name="expp", bufs=3) as xpp,
            tc.tile_pool(name="ounp", bufs=3) as onp,
            tc.tile_pool(name="rinvp", bufs=2) as rip,
            tc.tile_pool(name="osbp", bufs=2) as osp,
            tc.tile_pool(name="misc", bufs=2) as msc,
            tc.tile_pool(name="flex", bufs=2, space="PSUM") as flx,
            tc.tile_pool(name="sps", bufs=2, space="PSUM") as sps,
            tc.tile_pool(name="ops", bufs=1, space="PSUM") as ops,
        ):
            qk_sb = pp.tile([128, 2 * NPAIR, T], BF16, tag="qk")
            v_sb = pp.tile([128, NKT, NHL, HD + 1], BF16, tag="v")
            cos_t = pp.tile([96, T], BF16, tag="cos")
            sin_t = pp.tile([96, T], BF16, tag="sin")
            tri_t = pp.tile([128, 2 * 128], BF16, tag="tri")
            e6_t = pp.tile([6, NPAIR * 128], F32R, tag="e6")
            rs6_t = pp.tile([6, TQ], F32, tag="rs6")
            rinv6_t = pp.tile([6, TQ], F32R, tag="rinv6")

            wqk_t = wp.tile([128, C // 128, 2 * CL], BF16, tag="wqk")
            wv_t = wp.tile([128, C // 128, CL], BF16, tag="wv")
            wo_t = wp.tile([128, NPAIR, C], BF16, tag="wo")

            xr = xt.rearrange("(c p) t -> p c t", p=128)
            wqk_r = wqkt.rearrange("(co p) m -> co p m", p=128)

            def dma_x(jt):
                x_jt = xlp.tile([128, C // 128, TQ], BF16, tag="x")
                ts = slice(jt * TQ, (jt + 1) * TQ)
                nc.scalar.dma_start(x_jt[:, 0:3], xr[:, 0:3, ts])
                nc.scalar.dma_start(x_jt[:, 3:6], xr[:, 3:6, ts])
                return x_jt

            # ---- prologue DMAs: x(0) + weights, then remaining x ----
            x_tiles = [dma_x(0)]
            for c in range(C // 128):
                nc.scalar.dma_start(wqk_t[:, c], wqk_r[c])
            nc.scalar.dma_start(
                wv_t, wvt.rearrange("(co p) m -> p co m", p=128))
            nc.sync.dma_start(cos_t, cosb[:, :])
            nc.sync.dma_start(sin_t, sinb[:, :])
            nc.sync.dma_start(tri_t, tri2[:, :])
            nc.sync.dma_start(e6_t, e6[:, :])
            for jt in range(1, NTQ):
                x_tiles.append(dma_x(jt))
            nc.gpsimd.memset(
                v_sb.bitcast(mybir.dt.uint16).rearrange("p a b c -> p (a b c)"),
                0x3F80)  # bf16 1.0 bit pattern

            def proj(jt, norm=None):
                """qkv projections for t-tile jt into qk_sb / v_sb.

                The x(jt+1) prefetch is issued at the END: every later
                consumer of a DGE ring waits on the ring tail at its issue
                time, so an early prefetch would drag the next x transfer
                into this tile's projection critical path."""
                ts = slice(jt * TQ, (jt + 1) * TQ)
                x_jt = x_tiles[jt]

                def pass_scatter(src_ps, mt):
                    r0 = 0 if mt > 1 else 64          # first pass row in tile
                    prow0 = 128 * mt + r0 - 192       # pass-row index of r0
                    stg = psg.tile([128, TQ], BF16, tag="pstg")
                    nc.vector.tensor_copy(stg[r0:128], src_ps[r0:128])
                    row = prow0
                    while row < prow0 + 128 - r0:
                        blk, part = pass_dest(row)
                        run = min(prow0 + 128 - r0 - row, 48 - row % 48)
                        sr = row - prow0 + r0
                        nc.sync.dma_start(
                            qk_sb[part:part + run, blk, ts],
                            stg[sr:sr + run])
                        row += run

                # pass tiles 2..5 FIRST: they use the flx pool, which is free
                # at proj start, so the projection never waits for the
                # previous q-tile's attention to evict its o accumulator.
                for mt in range(2, 6):
                    ps = flx.tile([128, TQ], F32, tag="flex")
                    for c in range(C // 128):
                        nc.tensor.matmul(
                            ps, wqk_t[:, c, 128 * mt:128 * (mt + 1)],
                            x_jt[:, c], start=(c == 0),
                            stop=(c == C // 128 - 1))
                    pass_scatter(ps, mt)
                    if mt == 2 and norm is not None:
                        norm()
                # rope M-tiles (0, 1): two flx psum tiles psA, psB
                psA = flx.tile([128, TQ], F32, tag="flex")
                psB = flx.tile([128, TQ], F32, tag="flex")
                for mt, psm in ((0, psA), (1, psB)):
                    for c in range(C // 128):
                        nc.tensor.matmul(
                            psm, wqk_t[:, c, 128 * mt:128 * (mt + 1)],
                            x_jt[:, c], start=(c == 0), stop=(c == C // 128 - 1))
                # rope: rot1 = r1*cos - r2*sin ; rot2 = r2*cos + r1*sin
                # rope rows use A_ORDER so the r2 psum split lands on the
                # 64-partition boundary (DVE APs must be 0/32/64/96-aligned):
                # r1 = psA[0:96]; r2 = psB[0:64] ++ psA[96:128]
                t1 = rtp.tile([96, TQ], F32, tag="t1")
                t2 = rtp.tile([96, TQ], F32, tag="t2")
                rot = rop.tile([96, 2, TQ], BF16, tag="rot")
                rot1 = rot[:, 0, :]
                rot2 = rot[:, 1, :]
                nc.vector.tensor_tensor(t1, psA[0:96], cos_t[:, ts], MUL)
                nc.vector.tensor_tensor(
                    t2[0:64], psB[0:64], sin_t[0:64, ts], MUL)
                nc.vector.tensor_tensor(
                    t2[64:96], psA[96:128], sin_t[64:96, ts], MUL)
                nc.vector.tensor_tensor(rot1, t1, t2, SUB)
                t3 = rtp.tile([96, TQ], F32, tag="t1")
                t4 = rtp.tile([96, TQ], F32, tag="t2")
                nc.vector.tensor_tensor(
                    t3[0:64], psB[0:64], cos_t[0:64, ts], MUL)
                nc.vector.tensor_tensor(
                    t3[64:96], psA[96:128], cos_t[64:96, ts], MUL)
                nc.vector.tensor_tensor(t4, psA[0:96], sin_t[:, ts], MUL)
                nc.vector.tensor_tensor(rot2, t3, t4, ADD)
                # tile 1 rows 64:128 are pass rows
                pass_scatter(psB, 1)
                # rope scatter: one DMA per tensor-head, pair-0 heads first;
                # src [8, 2, TQ] free dims flatten into 16 dst partitions,
                # interleaving (r1_j, r2_j) pairs within the head's rope dims.
                for a in (0, 1, 6, 7, 2, 3, 8, 9, 4, 5, 10, 11):
                    i = A_ORDER.index(a)
                    tn, hl = divmod(a, NHL)
                    blk = (0 if tn == 0 else NPAIR) + hl // 2
                    base = 64 * (hl % 2)
                    nc.sync.dma_start(qk_sb[base:base + 16, blk, ts],
                                      rot[8 * i:8 * i + 8, :, :])
                # v projection: pv[tok, chan] per 128-token chunk
                for vt in range(TQ // 128):
                    pvf = flx.tile([128, TQ], F32, tag="flex")
                    pv = pvf[:, 0:CL]
                    kt0 = jt * (TQ // 128) + vt
                    for c in range(C // 128):
                        nc.tensor.matmul(
                            pv, x_jt[:, c, vt * 128:(vt + 1) * 128],
                            wv_t[:, c], start=(c == 0), stop=(c == C // 128 - 1))
                    nc.vector.tensor_copy(
                        v_sb[:, kt0, :, 0:HD],
                        pv.rearrange("p (h d) -> p h d", d=HD))
                if jt == 0:
                    nc.scalar.dma_start(
                        wo_t, wot.rearrange("(po p) m -> p po m", p=128))

            def attn(jq):
                """causal attention for q-tile jq; writes o_sb (bf16)."""
                qs = slice(jq * TQ, (jq + 1) * TQ)
                o_sb = osp.tile([128, NPAIR, TQ], BF16, tag="osb")
                oun_all = onp.tile([128, NPAIR, 2, TQ], F32, tag="oun")
                nkt = 4 * (jq + 1)
                for p in range(NPAIR):
                    qb = qk_sb[:, p, qs]
                    kb = qk_sb[:, NPAIR + p, :]
                    o_ps = ops.tile([128, 2, TQ], F32, tag="o")
                    pend = []

                    def issue_av(kt, ep, a):
                        for h in range(2):
                            nc.tensor.matmul(
                                o_ps[0:HD + 1, h, a:TQ],
                                v_sb[:, kt, 2 * p + h, :], ep[:, h, a:TQ],
                                start=(kt == 0), stop=(kt == nkt - 1))

                    for kt in range(nkt):
                        m = kt - 4 * jq
                        a = 0 if m < 0 else 128 * m
                        ks = slice(kt * 128, (kt + 1) * 128)
                        sg = sps.tile([128, 2, TQ], F32, tag="s")
                        nc.tensor.matmul(
                            sg[:, 0, a:TQ], kb[0:64, ks], qb[0:64, a:TQ],
                            start=True, stop=True, tile_position=(0, 0))
                        nc.tensor.matmul(
                            sg[:, 1, a:TQ], kb[64:128, ks], qb[64:128, a:TQ],
                            start=True, stop=True, tile_position=(64, 0))
                        ep = xpp.tile([128, 2, TQ], BF16, tag="e")
                        nc.scalar.activation(ep[:, :, a:TQ], sg[:, :, a:TQ],
                                             AF.Exp, scale=0.125)
                        if m >= 0:
                            nc.gpsimd.tensor_tensor(
                                ep[:, :, a:a + 128], ep[:, :, a:a + 128],
                                tri_t.rearrange("p (h q) -> p h q", h=2), MUL)
                        pend.append((kt, ep, a))
                        if len(pend) > 2:
                            issue_av(*pend.pop(0))
                    for job in pend:
                        issue_av(*job)
                    # softmax rowsums (ones-column of v) -> oun row 64;
                    # gather to rs6 per pair (cheap 2-descriptor DMA) so the
                    # slow DVE reciprocal runs ONCE per jq on [6, TQ] (free
                    # size, not partition count, sets reciprocal cost).
                    if jq == NTQ - 1 and p == NPAIR - 1:
                        nc.vector.tensor_copy(oun_all[0:HD + 1, p],
                                              o_ps[0:HD + 1])
                    else:
                        nc.scalar.copy(oun_all[0:HD + 1, p], o_ps[0:HD + 1])
                dma = nc.sync if jq == NTQ - 1 else nc.scalar
                dma.dma_start(rs6_t, oun_all[HD:HD + 1, :, :, :])
                return o_sb, oun_all

            def outproj(jq, o_sb, oun_all):
                # normalization runs here, AFTER proj(jt+1) on the PE queue,
                # so the rowsum-gather -> reciprocal chain hides under the
                # projection matmuls instead of stalling bc.
                qs = slice(jq * TQ, (jq + 1) * TQ)
                for p in range(NPAIR):
                    bc = flx.tile([128, TQ], F32, tag="flex")
                    nc.tensor.matmul(bc, e6_t[:, p * 128:(p + 1) * 128],
                                     rinv6_t, start=True, stop=True)
                    nc.vector.tensor_tensor(
                        o_sb[0:64, p], oun_all[0:64, p, 0], bc[0:64], MUL)
                    nc.vector.tensor_tensor(
                        o_sb[64:128, p], oun_all[0:64, p, 1], bc[64:128], MUL)
                ost = msc.tile([128, C // 128, TQ], F32, tag="ost")
                for dt in range(C // 128):
                    po = flx.tile([128, TQ], F32, tag="flex")
                    for p in range(NPAIR):
                        nc.tensor.matmul(
                            po, wo_t[:, p, dt * 128:(dt + 1) * 128],
                            o_sb[:, p], start=(p == 0), stop=(p == NPAIR - 1))
                    nc.vector.tensor_copy(ost[:, dt], po)
                nc.scalar.dma_start(
                    out.rearrange("(do p) t -> p do t", p=128)[:, :, qs], ost)

            def norm():
                # reciprocal cost scales with free size; [6, TQ] batches all
                # pairs. Hoisted ahead of the rope DVE chain so bc never
                # queues behind proj's DVE work in the FIFO.
                with nc.allow_low_precision(reason="f32r storage is 32-bit"):
                    nc.vector.reciprocal(rinv6_t, rs6_t)

            proj(0)
            for jt in range(NTQ):
                o_sb, oun_all = attn(jt)
                if jt + 1 < NTQ:
                    proj(jt + 1, norm)
                else:
                    norm()
                outproj(jt, o_sb, oun_all)

    nc.compile()
    return nc


def _host_inputs(x, w_qkv, w_out):
    """Build per-core input dicts. Core i: batch i//2, head-group i%2."""
    import ml_dtypes

    BF = ml_dtypes.bfloat16
    xf = np.asarray(x, dtype=np.float32)
    w3 = np.asarray(w_qkv, dtype=np.float32).reshape(3, NH, HD, C)
    wo = np.asarray(w_out, dtype=np.float32)

    per_group = []
    for g in range(2):
        hs = range(g * NHL, (g + 1) * NHL)
        A_ORDER = [0, 2, 4, 1, 3, 5, 6, 8, 10, 7, 9, 11]
        rows = []
        # M-tile 0: r1 (A_ORDER tensor-heads x dims 0:8) + r2a (last 4 of
        # A_ORDER x dims 8:16); M-tile 1 rows 0:64: r2b (first 8 x 8:16)
        for a in A_ORDER:
            tn, hl = divmod(a, NHL)
            rows.append(w3[tn, g * NHL + hl, 0:8])
        for a in A_ORDER[8:12]:
            tn, hl = divmod(a, NHL)
            rows.append(w3[tn, g * NHL + hl, 8:16])
        for a in A_ORDER[0:8]:
            tn, hl = divmod(a, NHL)
            rows.append(w3[tn, g * NHL + hl, 8:16])
        # pass rows: blocks in BLK_ORDER; per blk h_even, h_odd
        for blk in (0, 3, 1, 4, 2, 5):
            tn, pr = divmod(blk, NPAIR)
            for ho in range(2):
                rows.append(w3[tn, g * NHL + 2 * pr + ho, 16:64])
        wqk = np.concatenate(rows, axis=0)                  # [768, C]
        wqkt = np.ascontiguousarray(wqk.T).astype(BF)       # [C, 768]
        wv = w3[2, list(hs)].reshape(CL, C)                 # [384, C]
        wvt = np.ascontiguousarray(wv.T).astype(BF)
        wotr = np.ascontiguousarray(
            wo[:, g * CL:(g + 1) * CL].T).astype(BF)        # [384, 768]
        per_group.append((wqkt, wvt, wotr))

    j = np.arange(RD // 2, dtype=np.float64)
    freqs = 1.0 / (10000.0 ** (2 * j / RD))
    t = np.arange(T, dtype=np.float64)
    ang = t[None, :] * freqs[:, None]                        # [8, T]
    cosb = np.ascontiguousarray(np.tile(np.cos(ang), (12, 1))).astype(BF)
    sinb = np.ascontiguousarray(np.tile(np.sin(ang), (12, 1))).astype(BF)

    kk = np.arange(128)[:, None]
    qq = np.arange(128)[None, :]
    tri = (kk <= qq).astype(BF)
    tri2 = np.ascontiguousarray(np.concatenate([tri, tri], axis=1))
    e6 = np.zeros((6, NPAIR * 128), dtype=np.float32)
    for p in range(NPAIR):
        e6[2 * p, p * 128:p * 128 + 64] = 1.0
        e6[2 * p + 1, p * 128 + 64:(p + 1) * 128] = 1.0

    in_maps = []
    for i in range(8):
        b, g = divmod(i, 2)
        wqkt, wvt, wotr = per_group[g]
        in_maps.append({
            "xt": np.ascontiguousarray(xf[b].T).astype(BF),
            "wqkt": wqkt, "wvt": wvt, "wot": wotr,
            "cosb": cosb, "sinb": sinb, "tri2": tri2, "e6": e6,
        })
    return in_maps


def kernel(x, w_qkv, w_out, _trace=False):
    from concourse.bass_utils import run_bass_kernel_spmd

    if "nc" not in _cache:
        _cache["nc"] = _build()
    nc = _cache["nc"]
    in_maps = _host_inputs(x, w_qkv, w_out)
    res = run_bass_kernel_spmd(nc, in_maps, core_ids=list(range(8)),
                               trace=_trace)
    _cache["last_result"] = res
    out = np.empty((B, T, C), dtype=np.float32)
    for b in range(B):
        acc = res.results[2 * b]["out"].astype(np.float32) + \
            res.results[2 * b + 1]["out"].astype(np.float32)
        out[b] = acc.T
    return out



# revision 6
# speedup vs baseline: 1.0776x; 1.0776x over previous
"""Causal multi-head attention block (qkv proj + partial RoPE + causal attn +
out proj) for Trainium2, distributed over 8 NeuronCores.

Sharding: core i handles batch b = i//2 and head-group g = i%2 (6 of 12 heads).
Each core computes a partial output projection (contraction over its 6 heads'
384 channels); the host sums the two head-group partials per batch.

v4 design notes (evolved from v3 254us via NTFF trace analysis):
  - v3 serialized phases per q-tile: [attn (ACT-exp paced, PE ~35% idle)] ->
    [proj jt+1 (PE dense, ACT idle)] -> [outproj (PE dense, ACT idle)].
    Engine busy floors measured at the sustained 2.0 GHz P-state: PE ~165us,
    ACT ~145us, DVE ~95us -> v4 GLOBALLY software-pipelines: proj(jt+1) and
    outproj(jt-1) are generators whose matmul chunks are pumped between
    attention kt-steps, filling PE's exp-wait bubbles and keeping ACT's exp
    queue fed through former proj windows.
  - psA/psB (rope psum pair) are allocated back-to-back with no pump point
    between them: a foreign PSUM alloc interleaved there would WAR-deadlock
    against the rope reads (flx pool bufs=2).
  - Rowsum reciprocal via reciprocal_approx_fast (single custom-DVE op,
    ~5x faster than InstReciprocal, 51 ULP -- irrelevant vs the 2e-2 gate);
    un-hideable tail chain shrinks by ~4us.
  - oun (o-accumulator psum eviction) copies moved ACT -> DVE: ACT is the
    attention pacer; DVE has slack.
  - rowsum gathers ride the (mostly idle) gpsimd DGE ring; out stores ride
    the DVE ring right behind the ost copies that produce them.
  - out stored bf16 (host sums the two head-group partials in f32): halves
    store traffic; adds ~0.1% error against the 2e-2 gate.
  - All matmuls bf16 (1 cyc/col); scores use 64-row tile_position pairs
    (h0/h64) which stream ~1.3x faster than serial; attn inner loop software
    pipelined with lag 2 so PE rarely waits on ACT exp.
  - DMA discipline unchanged from v3: all x tiles + weights load up front on
    the ACT ring; SP ring carries only qk_sb scatters; batched single-DMA
    forms for x, weights, rowsums, output.
"""

import numpy as np

B, T, C = 4, 2048, 768
NH, HD, RD = 12, 64, 16
NHL = NH // 2          # heads per core (local)
NPAIR = NHL // 2       # head pairs per core
CL = NHL * HD          # local channels (384)
TQ = 512               # q tile
NTQ = T // TQ
NKT = T // 128         # k tiles of 128

_cache = {}


def _build(debug=False):
    import concourse.bacc as bacc
    import concourse.mybir as mybir
    import concourse.tile as tile

    F32R = mybir.dt.float32r
    F32 = mybir.dt.float32
    BF16 = mybir.dt.bfloat16
    AF = mybir.ActivationFunctionType
    MUL = mybir.AluOpType.mult
    SUB = mybir.AluOpType.subtract
    ADD = mybir.AluOpType.add

    nc = bacc.Bacc(trn_type="TRN2", name="attn8v4")

    xt = nc.dram_tensor("xt", [C, T], BF16, kind="ExternalInput")
    wqkt = nc.dram_tensor("wqkt", [C, 2 * CL], BF16, kind="ExternalInput")
    wvt = nc.dram_tensor("wvt", [C, CL], BF16, kind="ExternalInput")
    wot = nc.dram_tensor("wot", [CL, C], BF16, kind="ExternalInput")
    cosb = nc.dram_tensor("cosb", [96, T], BF16, kind="ExternalInput")
    sinb = nc.dram_tensor("sinb", [96, T], BF16, kind="ExternalInput")
    tri2 = nc.dram_tensor("tri2", [128, 2 * 128], BF16, kind="ExternalInput")
    e6 = nc.dram_tensor("e6", [6, NPAIR * 128], F32R, kind="ExternalInput")
    out = nc.dram_tensor("out", [C, T], BF16, kind="ExternalOutput")

    # qk-projection M-tiles (wqkt column order, host-built):
    #   tile 0 [128] : r1 rows [96] = (q h0..h5 | k h0..h5) x dims 0:8
    #                  + r2a rows [32] = tensor-heads 0..3 x dims 8:16
    #   tile 1 [128] : r2b rows [64] = tensor-heads 4..11 x dims 8:16
    #                  + pass rows 0:64
    #   tiles 2..5   : pass rows 64:576
    # pass row order: for blk 0..5 (q pairs then k pairs):
    #   h_even dims 16:64 (48 rows), h_odd dims 16:64 (48 rows)
    A_ORDER = [0, 2, 4, 1, 3, 5, 6, 8, 10, 7, 9, 11]

    BLK_ORDER = [0, 3, 1, 4, 2, 5]

    def pass_dest(row):
        g, r = divmod(row, 96)
        part = 64 * (r // 48) + 16 + (r % 48)
        return BLK_ORDER[g], part

    with tile.TileContext(nc) as tc:
        with (
            tc.tile_pool(name="persist", bufs=1) as pp,
            tc.tile_pool(name="weights", bufs=1) as wp,
            tc.tile_pool(name="xload", bufs=4) as xlp,
            tc.tile_pool(name="pstage", bufs=2) as psg,
            tc.tile_pool(name="ropet", bufs=1) as rtp,
            tc.tile_pool(name="rots", bufs=2) as rop,
            tc.tile_pool(name="expp", bufs=3) as xpp,
            tc.tile_pool(name="ounp", bufs=2) as onp,
            tc.tile_pool(name="osbp", bufs=2) as osp,
            tc.tile_pool(name="misc", bufs=2) as msc,
            tc.tile_pool(name="flex", bufs=2, space="PSUM") as flx,
            tc.tile_pool(name="sps", bufs=2, space="PSUM") as sps,
            tc.tile_pool(name="ops", bufs=1, space="PSUM") as ops,
        ):
            qk_sb = pp.tile([128, 2 * NPAIR, T], BF16, tag="qk")
            v_sb = pp.tile([128, NKT, NHL, HD + 1], BF16, tag="v")
            cos_t = pp.tile([96, T], BF16, tag="cos")
            sin_t = pp.tile([96, T], BF16, tag="sin")
            tri_t = pp.tile([128, 2 * 128], BF16, tag="tri")
            e6_t = pp.tile([6, NPAIR * 128], F32R, tag="e6")
            rs6_t = pp.tile([6, TQ], F32, tag="rs6")
            rf6_t = pp.tile([6, TQ], F32, tag="rf6")
            rinv6_t = pp.tile([6, TQ], F32R, tag="rinv6")

            wqk_t = wp.tile([128, C // 128, 2 * CL], BF16, tag="wqk")
            wv_t = wp.tile([128, C // 128, CL], BF16, tag="wv")
            wo_t = wp.tile([128, NPAIR, C], BF16, tag="wo")

            xr = xt.rearrange("(c p) t -> p c t", p=128)
            wqk_r = wqkt.rearrange("(co p) m -> co p m", p=128)
            out_r = out.rearrange("(do p) t -> p do t", p=128)

            def dma_x(jt):
                x_jt = xlp.tile([128, C // 128, TQ], BF16, tag="x")
                ts = slice(jt * TQ, (jt + 1) * TQ)
                nc.scalar.dma_start(x_jt[:, 0:3], xr[:, 0:3, ts])
                nc.scalar.dma_start(x_jt[:, 3:6], xr[:, 3:6, ts])
                return x_jt

            # ---- prologue DMAs: x(0) + weights, then remaining x ----
            x_tiles = [dma_x(0)]
            for c in range(C // 128):
                nc.scalar.dma_start(wqk_t[:, c], wqk_r[c])
            nc.sync.dma_start(
                wv_t, wvt.rearrange("(co p) m -> p co m", p=128))
            nc.sync.dma_start(cos_t, cosb[:, :])
            nc.sync.dma_start(sin_t, sinb[:, :])
            nc.sync.dma_start(tri_t, tri2[:, :])
            nc.sync.dma_start(e6_t, e6[:, :])
            for jt in range(1, NTQ):
                x_tiles.append(dma_x(jt))
            nc.gpsimd.memset(
                v_sb.bitcast(mybir.dt.uint16).rearrange("p a b c -> p (a b c)"),
                0x3F80)  # bf16 1.0 bit pattern

            def proj_gen(jt):
                """qkv projections for t-tile jt into qk_sb / v_sb.

                Generator: yields at chunk boundaries where the main loop may
                interleave attention kt-steps (and other PSUM pool users).
                The x prefetch ordering note from v3 still holds: later
                consumers of a DGE ring wait on the ring tail at issue time.
                """
                ts = slice(jt * TQ, (jt + 1) * TQ)
                x_jt = x_tiles[jt]

                def pass_scatter(src_ps, mt):
                    r0 = 0 if mt > 1 else 64          # first pass row in tile
                    prow0 = 128 * mt + r0 - 192       # pass-row index of r0
                    stg = psg.tile([128, TQ], BF16, tag="pstg")
                    nc.vector.tensor_copy(stg[r0:128], src_ps[r0:128])
                    row = prow0
                    while row < prow0 + 128 - r0:
                        blk, part = pass_dest(row)
                        run = min(prow0 + 128 - r0 - row, 48 - row % 48)
                        sr = row - prow0 + r0
                        nc.sync.dma_start(
                            qk_sb[part:part + run, blk, ts],
                            stg[sr:sr + run])
                        row += run

                # pass tiles 2..5 first (flx pool free at proj start)
                for mt in range(2, 6):
                    ps = flx.tile([128, TQ], F32, tag="flex")
                    for c in range(C // 128):
                        nc.tensor.matmul(
                            ps, wqk_t[:, c, 128 * mt:128 * (mt + 1)],
                            x_jt[:, c], start=(c == 0),
                            stop=(c == C // 128 - 1))
                        if c == 2:
                            yield
                    pass_scatter(ps, mt)
                    yield
                # rope M-tiles (0, 1): psA/psB allocated back-to-back with NO
                # yield between (a foreign flx alloc between them would
                # WAR-deadlock against the rope reads).
                psA = flx.tile([128, TQ], F32, tag="flex")
                psB = flx.tile([128, TQ], F32, tag="flex")
                for mt, psm in ((0, psA), (1, psB)):
                    for c in range(C // 128):
                        nc.tensor.matmul(
                            psm, wqk_t[:, c, 128 * mt:128 * (mt + 1)],
                            x_jt[:, c], start=(c == 0), stop=(c == C // 128 - 1))
                        if c == 2 and mt == 1:
                            yield
                yield
                # rope: rot1 = r1*cos - r2*sin ; rot2 = r2*cos + r1*sin
                # r1 = psA[0:96]; r2 = psB[0:64] ++ psA[96:128] (A_ORDER
                # packing keeps DVE APs 0/32/64/96-aligned)
                t1 = rtp.tile([96, TQ], F32, tag="t1")
                t2 = rtp.tile([96, TQ], F32, tag="t2")
                rot = rop.tile([96, 2, TQ], BF16, tag="rot")
                rot1 = rot[:, 0, :]
                rot2 = rot[:, 1, :]
                nc.vector.tensor_tensor(t1, psA[0:96], cos_t[:, ts], MUL)
                nc.vector.tensor_tensor(
                    t2[0:64], psB[0:64], sin_t[0:64, ts], MUL)
                nc.vector.tensor_tensor(
                    t2[64:96], psA[96:128], sin_t[64:96, ts], MUL)
                nc.vector.tensor_tensor(rot1, t1, t2, SUB)
                t3 = rtp.tile([96, TQ], F32, tag="t1")
                t4 = rtp.tile([96, TQ], F32, tag="t2")
                nc.vector.tensor_tensor(
                    t3[0:64], psB[0:64], cos_t[0:64, ts], MUL)
                nc.vector.tensor_tensor(
                    t3[64:96], psA[96:128], cos_t[64:96, ts], MUL)
                nc.vector.tensor_tensor(t4, psA[0:96], sin_t[:, ts], MUL)
                nc.vector.tensor_tensor(rot2, t3, t4, ADD)
                # tile 1 rows 64:128 are pass rows
                pass_scatter(psB, 1)
                yield
                # rope scatter: one DMA per tensor-head, pair-0 heads first;
                # src [8, 2, TQ] free dims flatten into 16 dst partitions,
                # interleaving (r1_j, r2_j) pairs within the head's rope dims.
                for a in (0, 1, 6, 7, 2, 3, 8, 9, 4, 5, 10, 11):
                    i = A_ORDER.index(a)
                    tn, hl = divmod(a, NHL)
                    blk = (0 if tn == 0 else NPAIR) + hl // 2
                    base = 64 * (hl % 2)
                    nc.sync.dma_start(qk_sb[base:base + 16, blk, ts],
                                      rot[8 * i:8 * i + 8, :, :])
                yield
                # v projection: pv[tok, chan] per 128-token chunk
                for vt in range(TQ // 128):
                    pvf = flx.tile([128, TQ], F32, tag="flex")
                    pv = pvf[:, 0:CL]
                    kt0 = jt * (TQ // 128) + vt
                    for c in range(C // 128):
                        nc.tensor.matmul(
                            pv, x_jt[:, c, vt * 128:(vt + 1) * 128],
                            wv_t[:, c], start=(c == 0), stop=(c == C // 128 - 1))
                        if c == 2:
                            yield
                    nc.vector.tensor_copy(
                        v_sb[:, kt0, :, 0:HD],
                        pv.rearrange("p (h d) -> p h d", d=HD))
                    yield
                if jt == 0:
                    nc.scalar.dma_start(
                        wo_t, wot.rearrange("(po p) m -> p po m", p=128))

            def outproj_gen(jq, oun_all):
                """normalize + output projection for q-tile jq (pumped inside
                attn(jq+1), or dense at the very end for the last tile)."""
                qs = slice(jq * TQ, (jq + 1) * TQ)
                o_sb = osp.tile([128, NPAIR, TQ], BF16, tag="osb")
                for p in range(NPAIR):
                    bc = flx.tile([128, TQ], F32, tag="flex")
                    nc.tensor.matmul(bc, e6_t[:, p * 128:(p + 1) * 128],
                                     rinv6_t, start=True, stop=True)
                    nc.vector.tensor_tensor(
                        o_sb[0:64, p], oun_all[0:64, p, 0], bc[0:64], MUL)
                    nc.vector.tensor_tensor(
                        o_sb[64:128, p], oun_all[0:64, p, 1], bc[64:128], MUL)
                    yield
                ost = msc.tile([128, C // 128, TQ], BF16, tag="ost")
                for dt in range(C // 128):
                    po = flx.tile([128, TQ], F32, tag="flex")
                    for p in range(NPAIR):
                        nc.tensor.matmul(
                            po, wo_t[:, p, dt * 128:(dt + 1) * 128],
                            o_sb[:, p], start=(p == 0), stop=(p == NPAIR - 1))
                    nc.vector.tensor_copy(ost[:, dt], po)
                    if dt == 2:
                        nc.gpsimd.dma_start(out_r[:, 0:3, qs], ost[:, 0:3])
                    yield
                nc.gpsimd.dma_start(out_r[:, 3:6, qs], ost[:, 3:6])

            def pump(gens):
                """advance the chunk generators by one chunk total."""
                while gens:
                    try:
                        next(gens[0])
                        return True
                    except StopIteration:
                        gens.pop(0)
                return False

            def attn(jq, gens, n_chunks):
                """causal attention for q-tile jq; interleaves `gens` chunks
                between kt-steps; writes oun_all (unnormalized o + rowsums)."""
                qs = slice(jq * TQ, (jq + 1) * TQ)
                oun_all = onp.tile([128, NPAIR, 2, TQ], F32, tag="oun")
                nkt = 4 * (jq + 1)
                per_step = n_chunks / (NPAIR * nkt)
                acc = 0.0
                for p in range(NPAIR):
                    qb = qk_sb[:, p, qs]
                    kb = qk_sb[:, NPAIR + p, :]
                    o_ps = ops.tile([128, 2, TQ], F32, tag="o")
                    pend = []

                    def issue_av(kt, ep, a):
                        for h in range(2):
                            nc.tensor.matmul(
                                o_ps[0:HD + 1, h, a:TQ],
                                v_sb[:, kt, 2 * p + h, :], ep[:, h, a:TQ],
                                start=(kt == 0), stop=(kt == nkt - 1))

                    for kt in range(nkt):
                        m = kt - 4 * jq
                        a = 0 if m < 0 else 128 * m
                        ks = slice(kt * 128, (kt + 1) * 128)
                        sg = sps.tile([128, 2, TQ], F32, tag="s")
                        nc.tensor.matmul(
                            sg[:, 0, a:TQ], kb[0:64, ks], qb[0:64, a:TQ],
                            start=True, stop=True, tile_position=(0, 0))
                        nc.tensor.matmul(
                            sg[:, 1, a:TQ], kb[64:128, ks], qb[64:128, a:TQ],
                            start=True, stop=True, tile_position=(64, 0))
                        ep = xpp.tile([128, 2, TQ], BF16, tag="e")
                        nc.scalar.activation(ep[:, :, a:TQ], sg[:, :, a:TQ],
                                             AF.Exp, scale=0.125)
                        if m >= 0:
                            nc.gpsimd.tensor_tensor(
                                ep[:, :, a:a + 128], ep[:, :, a:a + 128],
                                tri_t.rearrange("p (h q) -> p h q", h=2), MUL)
                        pend.append((kt, ep, a))
                        if len(pend) > 2:
                            issue_av(*pend.pop(0))
                        acc += per_step
                        if p == 0 and kt < 3:
                            # let the window-boundary rowsum->reciprocal chain
                            # land before the first outproj chunk (its bc
                            # matmul reads rinv6) can head-of-line block PE.
                            continue
                        while acc >= 1.0:
                            pump(gens)
                            acc -= 1.0
                    for job in pend:
                        issue_av(*job)
                    # evict o accumulator (+ rowsums in row HD) on DVE: ACT
                    # is the attention pacer, DVE has slack.
                    nc.vector.tensor_copy(oun_all[0:HD + 1, p],
                                          o_ps[0:HD + 1])
                # rowsum gather on the (idle) gpsimd DGE ring, then the fast
                # custom-DVE reciprocal (51 ULP; tolerance gate is 2e-2).
                nc.gpsimd.dma_start(rs6_t, oun_all[HD:HD + 1, :, :, :])
                nc.vector.reciprocal_approx_fast(rf6_t[:, :], rs6_t[:, :])
                with nc.allow_low_precision(reason="f32r storage is 32-bit"):
                    nc.vector.tensor_copy(rinv6_t, rf6_t)
                return oun_all

            # ---- main pipeline ----
            # prologue: proj(0) dense
            for _ in proj_gen(0):
                pass
            ouns = [None] * NTQ
            for jq in range(NTQ):
                gens = []
                n_chunks = 0
                if jq >= 1:
                    gens.append(outproj_gen(jq - 1, ouns[jq - 1]))
                    n_chunks += 9
                if jq + 1 < NTQ:
                    gens.append(proj_gen(jq + 1))
                    n_chunks += 22
                ouns[jq] = attn(jq, gens, n_chunks)
                while pump(gens):   # exhaust before next q-tile
                    pass
            # tail: last tile's normalize + out projection, dense
            for _ in outproj_gen(NTQ - 1, ouns[NTQ - 1]):
                pass

    nc.compile()
    return nc


def _host_inputs(x, w_qkv, w_out):
    """Build per-core input dicts. Core i: batch i//2, head-group i%2."""
    import ml_dtypes

    BF = ml_dtypes.bfloat16
    xf = np.asarray(x, dtype=np.float32)
    w3 = np.asarray(w_qkv, dtype=np.float32).reshape(3, NH, HD, C)
    wo = np.asarray(w_out, dtype=np.float32)

    per_group = []
    for g in range(2):
        hs = range(g * NHL, (g + 1) * NHL)
        A_ORDER = [0, 2, 4, 1, 3, 5, 6, 8, 10, 7, 9, 11]
        rows = []
        # M-tile 0: r1 (A_ORDER tensor-heads x dims 0:8) + r2a (last 4 of
        # A_ORDER x dims 8:16); M-tile 1 rows 0:64: r2b (first 8 x 8:16)
        for a in A_ORDER:
            tn, hl = divmod(a, NHL)
            rows.append(w3[tn, g * NHL + hl, 0:8])
        for a in A_ORDER[8:12]:
            tn, hl = divmod(a, NHL)
            rows.append(w3[tn, g * NHL + hl, 8:16])
        for a in A_ORDER[0:8]:
            tn, hl = divmod(a, NHL)
            rows.append(w3[tn, g * NHL + hl, 8:16])
        # pass rows: blocks in BLK_ORDER; per blk h_even, h_odd
        for blk in (0, 3, 1, 4, 2, 5):
            tn, pr = divmod(blk, NPAIR)
            for ho in range(2):
                rows.append(w3[tn, g * NHL + 2 * pr + ho, 16:64])
        wqk = np.concatenate(rows, axis=0)                  # [768, C]
        wqkt = np.ascontiguousarray(wqk.T).astype(BF)       # [C, 768]
        wv = w3[2, list(hs)].reshape(CL, C)                 # [384, C]
        wvt = np.ascontiguousarray(wv.T).astype(BF)
        wotr = np.ascontiguousarray(
            wo[:, g * CL:(g + 1) * CL].T).astype(BF)        # [384, 768]
        per_group.append((wqkt, wvt, wotr))

    j = np.arange(RD // 2, dtype=np.float64)
    freqs = 1.0 / (10000.0 ** (2 * j / RD))
    t = np.arange(T, dtype=np.float64)
    ang = t[None, :] * freqs[:, None]                        # [8, T]
    cosb = np.ascontiguousarray(np.tile(np.cos(ang), (12, 1))).astype(BF)
    sinb = np.ascontiguousarray(np.tile(np.sin(ang), (12, 1))).astype(BF)

    kk = np.arange(128)[:, None]
    qq = np.arange(128)[None, :]
    tri = (kk <= qq).astype(BF)
    tri2 = np.ascontiguousarray(np.concatenate([tri, tri], axis=1))
    e6 = np.zeros((6, NPAIR * 128), dtype=np.float32)
    for p in range(NPAIR):
        e6[2 * p, p * 128:p * 128 + 64] = 1.0
        e6[2 * p + 1, p * 128 + 64:(p + 1) * 128] = 1.0

    in_maps = []
    for i in range(8):
        b, g = divmod(i, 2)
        wqkt, wvt, wotr = per_group[g]
        in_maps.append({
            "xt": np.ascontiguousarray(xf[b].T).astype(BF),
            "wqkt": wqkt, "wvt": wvt, "wot": wotr,
            "cosb": cosb, "sinb": sinb, "tri2": tri2, "e6": e6,
        })
    return in_maps


def kernel(x, w_qkv, w_out, _trace=False):
    from concourse.bass_utils import run_bass_kernel_spmd

    if "nc" not in _cache:
        _cache["nc"] = _build()
    nc = _cache["nc"]
    in_maps = _host_inputs(x, w_qkv, w_out)
    res = run_bass_kernel_spmd(nc, in_maps, core_ids=list(range(8)),
                               trace=_trace)
    _cache["last_result"] = res
    out = np.empty((B, T, C), dtype=np.float32)
    for b in range(B):
        acc = res.results[2 * b]["out"].astype(np.float32) + \
            res.results[2 * b + 1]["out"].astype(np.float32)
        out[b] = acc.T
    return out


# revision 14
# speedup vs baseline: 1.2818x; 1.1895x over previous
"""Causal multi-head attention block (qkv proj + partial RoPE + causal attn +
out proj) for Trainium2, distributed over 8 NeuronCores.

Sharding: core i handles batch b = i//2 and head-group g = i%2 (6 of 12 heads).
Each core computes a partial output projection (contraction over its 6 heads'
384 channels); the host sums the two head-group partials per batch.

v4 design notes (evolved from v3 254us via NTFF trace analysis):
  - v3 serialized phases per q-tile: [attn (ACT-exp paced, PE ~35% idle)] ->
    [proj jt+1 (PE dense, ACT idle)] -> [outproj (PE dense, ACT idle)].
    Engine busy floors measured at the sustained 2.0 GHz P-state: PE ~165us,
    ACT ~145us, DVE ~95us -> v4 GLOBALLY software-pipelines: proj(jt+1) and
    outproj(jt-1) are generators whose matmul chunks are pumped between
    attention kt-steps, filling PE's exp-wait bubbles and keeping ACT's exp
    queue fed through former proj windows.
  - psA/psB (rope psum pair) are allocated back-to-back with no pump point
    between them: a foreign PSUM alloc interleaved there would WAR-deadlock
    against the rope reads (flx pool bufs=2).
  - Rowsum reciprocal via reciprocal_approx_fast (single custom-DVE op,
    ~5x faster than InstReciprocal, 51 ULP -- irrelevant vs the 2e-2 gate);
    un-hideable tail chain shrinks by ~4us.
  - oun (o-accumulator psum eviction) copies moved ACT -> DVE: ACT is the
    attention pacer; DVE has slack.
  - rowsum gathers ride the (mostly idle) gpsimd DGE ring; out stores ride
    the DVE ring right behind the ost copies that produce them.
  - out stored bf16 (host sums the two head-group partials in f32): halves
    store traffic; adds ~0.1% error against the 2e-2 gate.
  - All matmuls bf16 (1 cyc/col); scores use 64-row tile_position pairs
    (h0/h64) which stream ~1.3x faster than serial; attn inner loop software
    pipelined with lag 2 so PE rarely waits on ACT exp.
  - DMA discipline unchanged from v3: all x tiles + weights load up front on
    the ACT ring; SP ring carries only qk_sb scatters; batched single-DMA
    forms for x, weights, rowsums, output.
"""

import numpy as np

B, T, C = 4, 2048, 768
NH, HD, RD = 12, 64, 16
NHL = NH // 2          # heads per core (local)
NPAIR = NHL // 2       # head pairs per core
CL = NHL * HD          # local channels (384)
TQ = 512               # q tile
NTQ = T // TQ
NKT = T // 128         # k tiles of 128

_cache = {}


def _build(debug=False):
    import concourse.bacc as bacc
    import concourse.mybir as mybir
    import concourse.tile as tile

    F32R = mybir.dt.float32r
    F32 = mybir.dt.float32
    BF16 = mybir.dt.bfloat16
    AF = mybir.ActivationFunctionType
    MUL = mybir.AluOpType.mult
    SUB = mybir.AluOpType.subtract
    ADD = mybir.AluOpType.add

    nc = bacc.Bacc(trn_type="TRN2", name="attn8v4")

    xt = nc.dram_tensor("xt", [C, T], BF16, kind="ExternalInput")
    wqkt = nc.dram_tensor("wqkt", [C, 2 * CL], BF16, kind="ExternalInput")
    wvt = nc.dram_tensor("wvt", [C, CL], BF16, kind="ExternalInput")
    wot = nc.dram_tensor("wot", [CL, C], BF16, kind="ExternalInput")
    cosb = nc.dram_tensor("cosb", [96, T], BF16, kind="ExternalInput")
    sinb = nc.dram_tensor("sinb", [96, T], BF16, kind="ExternalInput")
    tri2 = nc.dram_tensor("tri2", [128, 2 * 128], BF16, kind="ExternalInput")
    e6 = nc.dram_tensor("e6", [6, NPAIR * 128], BF16, kind="ExternalInput")
    out = nc.dram_tensor("out", [C, T], BF16, kind="ExternalOutput")

    # qk-projection M-tiles (wqkt column order, host-built):
    #   tile 0 [128] : r1 rows [96] = (q h0..h5 | k h0..h5) x dims 0:8
    #                  + r2a rows [32] = tensor-heads 0..3 x dims 8:16
    #   tile 1 [128] : r2b rows [64] = tensor-heads 4..11 x dims 8:16
    #                  + pass rows 0:64
    #   tiles 2..5   : pass rows 64:576
    # pass row order: for blk 0..5 (q pairs then k pairs):
    #   h_even dims 16:64 (48 rows), h_odd dims 16:64 (48 rows)
    A_ORDER = [0, 2, 4, 1, 3, 5, 6, 8, 10, 7, 9, 11]

    BLK_ORDER = [0, 3, 1, 4, 2, 5]

    def pass_dest(row):
        g, r = divmod(row, 96)
        part = 64 * (r // 48) + 16 + (r % 48)
        return BLK_ORDER[g], part

    with tile.TileContext(nc) as tc:
        with (
            tc.tile_pool(name="persist", bufs=1) as pp,
            tc.tile_pool(name="weights", bufs=1) as wp,
            tc.tile_pool(name="xload", bufs=4) as xlp,
            tc.tile_pool(name="pstage", bufs=2) as psg,
            tc.tile_pool(name="ropet", bufs=1) as rtp,
            tc.tile_pool(name="rots", bufs=2) as rop,
            tc.tile_pool(name="expp", bufs=3) as xpp,
            tc.tile_pool(name="ounp", bufs=2) as onp,
            tc.tile_pool(name="osbp", bufs=2) as osp,
            tc.tile_pool(name="misc", bufs=2) as msc,
            tc.tile_pool(name="flex", bufs=2, space="PSUM") as flx,
            tc.tile_pool(name="sps", bufs=2, space="PSUM") as sps,
            tc.tile_pool(name="ops", bufs=1, space="PSUM") as ops,
        ):
            qk_sb = pp.tile([128, 2 * NPAIR, T], BF16, tag="qk")
            v_sb = pp.tile([128, NKT, NHL, HD + 1], BF16, tag="v")
            cos_t = pp.tile([96, T], BF16, tag="cos")
            sin_t = pp.tile([96, T], BF16, tag="sin")
            tri_t = pp.tile([128, 2 * 128], BF16, tag="tri")
            e6_t = pp.tile([6, NPAIR * 128], BF16, tag="e6")
            rs6_t = pp.tile([6, TQ], F32, tag="rs6")
            rf6_t = pp.tile([6, TQ], F32, tag="rf6")
            rinv6_t = pp.tile([6, TQ], BF16, tag="rinv6")

            wqk_t = wp.tile([128, C // 128, 2 * CL], BF16, tag="wqk")
            wv_t = wp.tile([128, C // 128, CL], BF16, tag="wv")
            wo_t = wp.tile([128, NPAIR, C], BF16, tag="wo")

            xr = xt.rearrange("(c p) t -> p c t", p=128)
            wqk_r = wqkt.rearrange("(co p) m -> co p m", p=128)
            out_r = out.rearrange("(do p) t -> p do t", p=128)

            def dma_x(jt):
                x_jt = xlp.tile([128, C // 128, TQ], BF16, tag="x")
                ts = slice(jt * TQ, (jt + 1) * TQ)
                nc.scalar.dma_start(x_jt[:, 0:3], xr[:, 0:3, ts])
                nc.scalar.dma_start(x_jt[:, 3:6], xr[:, 3:6, ts])
                return x_jt

            # ---- prologue DMAs: x(0) + weights split across both rings so
            # the first (rope) matmuls can start as early as possible ----
            x_tiles = [dma_x(0)]
            for c in range(3):
                nc.scalar.dma_start(wqk_t[:, c], wqk_r[c])
            for c in range(3, C // 128):
                nc.sync.dma_start(wqk_t[:, c], wqk_r[c])
            nc.sync.dma_start(cos_t, cosb[:, :])
            nc.sync.dma_start(sin_t, sinb[:, :])
            nc.sync.dma_start(
                wv_t, wvt.rearrange("(co p) m -> p co m", p=128))
            nc.sync.dma_start(tri_t, tri2[:, :])
            nc.sync.dma_start(e6_t, e6[:, :])
            for jt in range(1, NTQ):
                x_tiles.append(dma_x(jt))
            nc.gpsimd.memset(
                v_sb.bitcast(mybir.dt.uint16).rearrange("p a b c -> p (a b c)"),
                0x3F80)  # bf16 1.0 bit pattern

            def proj_gen(jt, rope_first=False):
                """qkv projections for t-tile jt into qk_sb / v_sb.

                Generator: yields at chunk boundaries where the main loop may
                interleave attention kt-steps (and other PSUM pool users).
                rope_first reorders for the prologue tile (jt=0) so attn(0)
                can start as early as possible; chunk indices for that order
                are documented at the call site (gates).
                """
                ts = slice(jt * TQ, (jt + 1) * TQ)
                x_jt = x_tiles[jt]

                def pass_scatter(src_ps, mt):
                    r0 = 0 if mt > 1 else 64          # first pass row in tile
                    prow0 = 128 * mt + r0 - 192       # pass-row index of r0
                    stg = psg.tile([128, TQ], BF16, tag="pstg")
                    nc.vector.tensor_copy(stg[r0:128], src_ps[r0:128])
                    row = prow0
                    while row < prow0 + 128 - r0:
                        blk, part = pass_dest(row)
                        run = min(prow0 + 128 - r0 - row, 48 - row % 48)
                        sr = row - prow0 + r0
                        nc.sync.dma_start(
                            qk_sb[part:part + run, blk, ts],
                            stg[sr:sr + run])
                        row += run

                def gen_pass(mt):
                    ps = flx.tile([128, TQ], F32, tag="flex")
                    for c in range(C // 128):
                        nc.tensor.matmul(
                            ps, wqk_t[:, c, 128 * mt:128 * (mt + 1)],
                            x_jt[:, c], start=(c == 0),
                            stop=(c == C // 128 - 1))
                        if c == 2:
                            yield
                    pass_scatter(ps, mt)
                    yield

                def gen_rope():
                    # rope M-tiles (0, 1): psA/psB allocated back-to-back
                    # with NO yield between (a foreign flx alloc interleaved
                    # there would WAR-deadlock against the rope reads).
                    psA = flx.tile([128, TQ], F32, tag="flex")
                    psB = flx.tile([128, TQ], F32, tag="flex")
                    for mt, psm in ((0, psA), (1, psB)):
                        for c in range(C // 128):
                            nc.tensor.matmul(
                                psm, wqk_t[:, c, 128 * mt:128 * (mt + 1)],
                                x_jt[:, c], start=(c == 0),
                                stop=(c == C // 128 - 1))
                            if c == 2 and mt == 1:
                                yield
                    yield
                    # rot1 = r1*cos - r2*sin ; rot2 = r2*cos + r1*sin
                    # r1 = psA[0:96]; r2 = psB[0:64] ++ psA[96:128]
                    t1 = rtp.tile([96, TQ], F32, tag="t1")
                    t2 = rtp.tile([96, TQ], F32, tag="t2")
                    rot = rop.tile([96, 2, TQ], BF16, tag="rot")
                    rot1 = rot[:, 0, :]
                    rot2 = rot[:, 1, :]
                    nc.vector.tensor_tensor(t1, psA[0:96], cos_t[:, ts], MUL)
                    nc.vector.tensor_tensor(
                        t2[0:64], psB[0:64], sin_t[0:64, ts], MUL)
                    nc.vector.tensor_tensor(
                        t2[64:96], psA[96:128], sin_t[64:96, ts], MUL)
                    nc.vector.tensor_tensor(rot1, t1, t2, SUB)
                    t3 = rtp.tile([96, TQ], F32, tag="t1")
                    t4 = rtp.tile([96, TQ], F32, tag="t2")
                    nc.vector.tensor_tensor(
                        t3[0:64], psB[0:64], cos_t[0:64, ts], MUL)
                    nc.vector.tensor_tensor(
                        t3[64:96], psA[96:128], cos_t[64:96, ts], MUL)
                    nc.vector.tensor_tensor(t4, psA[0:96], sin_t[:, ts], MUL)
                    nc.vector.tensor_tensor(rot2, t3, t4, ADD)
                    # tile 1 rows 64:128 are pass rows
                    pass_scatter(psB, 1)
                    yield
                    # rope scatter: one DMA per tensor-head; src [8, 2, TQ]
                    # free dims flatten into 16 dst partitions.
                    for a in (0, 1, 6, 7, 2, 3, 8, 9, 4, 5, 10, 11):
                        i = A_ORDER.index(a)
                        tn, hl = divmod(a, NHL)
                        blk = (0 if tn == 0 else NPAIR) + hl // 2
                        base = 64 * (hl % 2)
                        nc.sync.dma_start(qk_sb[base:base + 16, blk, ts],
                                          rot[8 * i:8 * i + 8, :, :])
                    yield

                def gen_v(vt):
                    pvf = flx.tile([128, TQ], F32, tag="flex")
                    pv = pvf[:, 0:CL]
                    kt0 = jt * (TQ // 128) + vt
                    for c in range(C // 128):
                        nc.tensor.matmul(
                            pv, x_jt[:, c, vt * 128:(vt + 1) * 128],
                            wv_t[:, c], start=(c == 0), stop=(c == C // 128 - 1))
                        if c == 2:
                            yield
                    nc.vector.tensor_copy(
                        v_sb[:, kt0, :, 0:HD],
                        pv.rearrange("p (h d) -> p h d", d=HD))
                    yield

                if rope_first:
                    parts = ([gen_rope(), gen_pass(2)] +
                             [gen_v(vt) for vt in range(TQ // 128)] +
                             [gen_pass(3), gen_pass(4), gen_pass(5)])
                else:
                    parts = ([gen_pass(mt) for mt in range(2, 6)] +
                             [gen_rope()] +
                             [gen_v(vt) for vt in range(TQ // 128)])
                for g in parts:
                    yield from g
                if jt == 0:
                    nc.scalar.dma_start(
                        wo_t, wot.rearrange("(po p) m -> p po m", p=128))

            def outproj_gen(jq, oun_all, tail=False):
                """normalize + output projection for q-tile jq (pumped inside
                attn(jq+1), or dense at the very end for the last tile).
                tail=True alternates the PSUM evictions DVE/ACT so the flx
                WAR chain isn't serialized on one engine's latency (ACT is
                idle after the last exp)."""
                qs = slice(jq * TQ, (jq + 1) * TQ)
                o_sb = osp.tile([128, NPAIR, TQ], BF16, tag="osb")
                for p in range(NPAIR):
                    bc = flx.tile([128, TQ], F32, tag="flex")
                    nc.tensor.matmul(bc, e6_t[:, p * 128:(p + 1) * 128],
                                     rinv6_t, start=True, stop=True)
                    nc.vector.tensor_tensor(
                        o_sb[0:64, p], oun_all[0:64, p, 0], bc[0:64], MUL)
                    nc.vector.tensor_tensor(
                        o_sb[64:128, p], oun_all[0:64, p, 1], bc[64:128], MUL)
                    yield
                ost = msc.tile([128, C // 128, TQ], BF16, tag="ost")
                for dt in range(C // 128):
                    po = flx.tile([128, TQ], F32, tag="flex")
                    for p in range(NPAIR):
                        nc.tensor.matmul(
                            po, wo_t[:, p, dt * 128:(dt + 1) * 128],
                            o_sb[:, p], start=(p == 0), stop=(p == NPAIR - 1))
                    if tail and dt % 2 == 1:
                        nc.scalar.copy(ost[:, dt], po)
                    else:
                        nc.vector.tensor_copy(ost[:, dt], po)
                    if dt == 2:
                        nc.gpsimd.dma_start(out_r[:, 0:3, qs], ost[:, 0:3])
                    yield
                nc.gpsimd.dma_start(out_r[:, 3:6, qs], ost[:, 3:6])

            cnt = [0]   # chunks pumped, global (gates index into this)

            def pump(gens):
                """advance the chunk generators by one chunk total."""
                while gens:
                    try:
                        next(gens[0])
                        cnt[0] += 1
                        return True
                    except StopIteration:
                        gens.pop(0)
                return False

            def attn(jq, gens, n_chunks, pair_gates=None, av_gate=None):
                """causal attention for q-tile jq; interleaves `gens` chunks
                between kt-steps; writes oun_all (unnormalized o + rowsums).

                pair_gates / av_gate (jq=0 only): minimum cnt[0] that must be
                pumped before emitting pair p's first score / av(kt) -- an
                attention instruction emitted before its producer chunks
                would head-of-line deadlock the in-order PE queue.
                """
                qs = slice(jq * TQ, (jq + 1) * TQ)
                oun_all = onp.tile([128, NPAIR, 2, TQ], F32, tag="oun")
                nkt = 4 * (jq + 1)
                per_step = n_chunks / (NPAIR * nkt)
                acc = 0.0

                def pump_until(k):
                    while cnt[0] < k and pump(gens):
                        pass

                for p in range(NPAIR):
                    if pair_gates:
                        pump_until(pair_gates[p])
                    qb = qk_sb[:, p, qs]
                    kb = qk_sb[:, NPAIR + p, :]
                    o_ps = ops.tile([128, 2, TQ], F32, tag="o")
                    pend = []

                    def issue_av(kt, ep, a):
                        if av_gate:
                            pump_until(av_gate(kt))
                        for h in range(2):
                            nc.tensor.matmul(
                                o_ps[0:HD + 1, h, a:TQ],
                                v_sb[:, kt, 2 * p + h, :], ep[:, h, a:TQ],
                                start=(kt == 0), stop=(kt == nkt - 1))

                    for kt in range(nkt):
                        m = kt - 4 * jq
                        a = 0 if m < 0 else 128 * m
                        ks = slice(kt * 128, (kt + 1) * 128)
                        sg = sps.tile([128, 2, TQ], F32, tag="s")
                        nc.tensor.matmul(
                            sg[:, 0, a:TQ], kb[0:64, ks], qb[0:64, a:TQ],
                            start=True, stop=True, tile_position=(0, 0))
                        nc.tensor.matmul(
                            sg[:, 1, a:TQ], kb[64:128, ks], qb[64:128, a:TQ],
                            start=True, stop=True, tile_position=(64, 0))
                        ep = xpp.tile([128, 2, TQ], BF16, tag="e")
                        nc.scalar.activation(ep[:, :, a:TQ], sg[:, :, a:TQ],
                                             AF.Exp, scale=0.125)
                        if m >= 0:
                            nc.gpsimd.tensor_tensor(
                                ep[:, :, a:a + 128], ep[:, :, a:a + 128],
                                tri_t.rearrange("p (h q) -> p h q", h=2), MUL)
                        pend.append((kt, ep, a))
                        if len(pend) > 2:
                            issue_av(*pend.pop(0))
                        acc += per_step
                        if jq > 0 and p == 0 and kt < 3:
                            # let the window-boundary rowsum->reciprocal chain
                            # land before the first outproj chunk (its bc
                            # matmul reads rinv6) can head-of-line block PE.
                            continue
                        while acc >= 1.0:
                            pump(gens)
                            acc -= 1.0
                    for job in pend:
                        issue_av(*job)
                    # evict o accumulator (+ rowsums in row HD) on DVE: ACT
                    # is the attention pacer, DVE has slack.
                    nc.vector.tensor_copy(oun_all[0:HD + 1, p],
                                          o_ps[0:HD + 1])
                # rowsum gather on the (idle) gpsimd DGE ring, then the fast
                # custom-DVE reciprocal (51 ULP; tolerance gate is 2e-2).
                nc.gpsimd.dma_start(rs6_t, oun_all[HD:HD + 1, :, :, :])
                nc.vector.reciprocal_approx_fast(rf6_t[:, :], rs6_t[:, :])
                nc.vector.tensor_copy(rinv6_t, rf6_t)
                return oun_all

            # ---- main pipeline ----
            # prologue: proj(0) rope part dense (chunks 1-4); the rest of
            # proj(0) is pumped inside attn(0) behind readiness gates.
            # proj(0) rope-first chunk map: rope 1-4, pass2 5-6, v0..v3
            # 7-8/9-10/11-12/13-14, pass3 15-16, pass4 17-18, pass5 19-20.
            g0 = proj_gen(0, rope_first=True)
            for _ in range(4):
                next(g0)
                cnt[0] += 1
            ouns = [None] * NTQ
            for jq in range(NTQ):
                gens = []
                n_chunks = 0
                if jq == 0:
                    gens.append(g0)
                    n_chunks += 16
                if jq >= 1:
                    gens.append(outproj_gen(jq - 1, ouns[jq - 1]))
                    n_chunks += 9
                if jq + 1 < NTQ:
                    gens.append(proj_gen(jq + 1))
                    n_chunks += 22
                pair_gates = [6, 18, 20] if jq == 0 else None
                av_gate = (lambda kt: 8 + 2 * kt) if jq == 0 else None
                ouns[jq] = attn(jq, gens, n_chunks, pair_gates, av_gate)
                while pump(gens):   # exhaust before next q-tile
                    pass
            # tail: last tile's normalize + out projection, dense
            for _ in outproj_gen(NTQ - 1, ouns[NTQ - 1], tail=True):
                pass

    nc.compile()
    return nc


def _host_inputs(x, w_qkv, w_out):
    """Build per-core input dicts. Core i: batch i//2, head-group i%2."""
    import ml_dtypes

    BF = ml_dtypes.bfloat16
    xf = np.asarray(x, dtype=np.float32)
    w3 = np.asarray(w_qkv, dtype=np.float32).reshape(3, NH, HD, C)
    wo = np.asarray(w_out, dtype=np.float32)

    per_group = []
    for g in range(2):
        hs = range(g * NHL, (g + 1) * NHL)
        A_ORDER = [0, 2, 4, 1, 3, 5, 6, 8, 10, 7, 9, 11]
        rows = []
        # M-tile 0: r1 (A_ORDER tensor-heads x dims 0:8) + r2a (last 4 of
        # A_ORDER x dims 8:16); M-tile 1 rows 0:64: r2b (first 8 x 8:16)
        for a in A_ORDER:
            tn, hl = divmod(a, NHL)
            rows.append(w3[tn, g * NHL + hl, 0:8])
        for a in A_ORDER[8:12]:
            tn, hl = divmod(a, NHL)
            rows.append(w3[tn, g * NHL + hl, 8:16])
        for a in A_ORDER[0:8]:
            tn, hl = divmod(a, NHL)
            rows.append(w3[tn, g * NHL + hl, 8:16])
        # pass rows: blocks in BLK_ORDER; per blk h_even, h_odd
        for blk in (0, 3, 1, 4, 2, 5):
            tn, pr = divmod(blk, NPAIR)
            for ho in range(2):
                rows.append(w3[tn, g * NHL + 2 * pr + ho, 16:64])
        wqk = np.concatenate(rows, axis=0)                  # [768, C]
        wqkt = np.ascontiguousarray(wqk.T).astype(BF)       # [C, 768]
        wv = w3[2, list(hs)].reshape(CL, C)                 # [384, C]
        wvt = np.ascontiguousarray(wv.T).astype(BF)
        wotr = np.ascontiguousarray(
            wo[:, g * CL:(g + 1) * CL].T).astype(BF)        # [384, 768]
        per_group.append((wqkt, wvt, wotr))

    j = np.arange(RD // 2, dtype=np.float64)
    freqs = 1.0 / (10000.0 ** (2 * j / RD))
    t = np.arange(T, dtype=np.float64)
    ang = t[None, :] * freqs[:, None]                        # [8, T]
    cosb = np.ascontiguousarray(np.tile(np.cos(ang), (12, 1))).astype(BF)
    sinb = np.ascontiguousarray(np.tile(np.sin(ang), (12, 1))).astype(BF)

    kk = np.arange(128)[:, None]
    qq = np.arange(128)[None, :]
    tri = (kk <= qq).astype(BF)
    tri2 = np.ascontiguousarray(np.concatenate([tri, tri], axis=1))
    e6 = np.zeros((6, NPAIR * 128), dtype=BF)
    for p in range(NPAIR):
        e6[2 * p, p * 128:p * 128 + 64] = 1.0
        e6[2 * p + 1, p * 128 + 64:(p + 1) * 128] = 1.0

    in_maps = []
    for i in range(8):
        b, g = divmod(i, 2)
        wqkt, wvt, wotr = per_group[g]
        in_maps.append({
            "xt": np.ascontiguousarray(xf[b].T).astype(BF),
            "wqkt": wqkt, "wvt": wvt, "wot": wotr,
            "cosb": cosb, "sinb": sinb, "tri2": tri2, "e6": e6,
        })
    return in_maps


def kernel(x, w_qkv, w_out, _trace=False):
    from concourse.bass_utils import run_bass_kernel_spmd

    if "nc" not in _cache:
        _cache["nc"] = _build()
    nc = _cache["nc"]
    in_maps = _host_inputs(x, w_qkv, w_out)
    res = run_bass_kernel_spmd(nc, in_maps, core_ids=list(range(8)),
                               trace=_trace)
    _cache["last_result"] = res
    out = np.empty((B, T, C), dtype=np.float32)
    for b in range(B):
        acc = res.results[2 * b]["out"].astype(np.float32) + \
            res.results[2 * b + 1]["out"].astype(np.float32)
        out[b] = acc.T
    return out


# revision 22
# speedup vs baseline: 1.3089x; 1.0211x over previous
"""Causal multi-head attention block (qkv proj + partial RoPE + causal attn +
out proj) for Trainium2, distributed over 8 NeuronCores.

Sharding: core i handles batch b = i//2 and head-group g = i%2 (6 of 12 heads).
Each core computes a partial output projection (contraction over its 6 heads'
384 channels); the host sums the two head-group partials per batch.

v6 design notes (evolved from v3 254us -> v5 228us-traced via NTFF analysis):
  - The kernel's wall time is the exp stream: 120 ACT Exp instructions
    (softmax) whose cols are fixed.  Everything else (PE proj/attn/outproj
    ~165us at the sustained 2.0 GHz P-state, DVE ~100us) must hide around
    it.  v6 pipelines globally: proj(jt) and outproj(jt) are generators
    whose chunks are pumped between attention kt-steps, with readiness
    GATES (see below) instead of phase barriers.
  - Gates: attention instructions are emitted only after the proj chunks
    producing their inputs have been emitted -- an instruction emitted
    before its producer would head-of-line deadlock the in-order PE queue.
    proj chunks spill across window boundaries (no dense drain between
    q-tiles, which cost ~9us of ACT idle in v4).
  - proj emits rope-first so the next window's scores unblock early.
    psA/psB (rope psum pair) are allocated back-to-back with no pump point
    between them (a foreign PSUM alloc there would WAR-deadlock the flx
    pool, bufs=2).
  - Prologue: 14 warm-up matmuls on a dummy tile keep the PE HAM activity
    window busy while x/weights stream in (cold K=4/8 halves the clock for
    the first ~3.4us of work); x is host-packed so its load is 2 fully
    contiguous 24KB/partition DMAs; v_sb ones-init memsets only the 96
    ones-columns (was 5.3us of gpsimd for the full tile).
  - DMA rings: SP carries pass scatters + half the rope scatters; the
    (otherwise idle) gpsimd ring carries the other rope half, rowsum
    gathers and output stores.  The ACT ring only loads x/weights so DMA
    issue never blocks the exp stream.
  - Tail (last q-tile): rowsums are gathered and inverted per-pair as each
    pair finishes, so normalize + the pair-0/1 output-projection matmuls
    overlap the remaining attention; PSUM evictions alternate DVE/ACT.
  - Rowsum reciprocal via reciprocal_approx_fast (custom DVE op, ~5x
    faster than InstReciprocal; 51 ULP vs the 2e-2 gate).  rinv/e6/bc are
    bf16 (exact 0/1 stationary; ~0.4% on the normalization, well in gate).
  - out is stored bf16 (host sums the two head-group partials in f32).
"""

import numpy as np

B, T, C = 4, 2048, 768
NH, HD, RD = 12, 64, 16
NHL = NH // 2          # heads per core (local)
NPAIR = NHL // 2       # head pairs per core
CL = NHL * HD          # local channels (384)
TQ = 512               # q tile
NTQ = T // TQ
NKT = T // 128         # k tiles of 128

_cache = {}


def _build(debug=False):
    import concourse.bacc as bacc
    import concourse.mybir as mybir
    import concourse.tile as tile

    F32 = mybir.dt.float32
    BF16 = mybir.dt.bfloat16
    U16 = mybir.dt.uint16
    AF = mybir.ActivationFunctionType
    MUL = mybir.AluOpType.mult
    SUB = mybir.AluOpType.subtract
    ADD = mybir.AluOpType.add

    nc = bacc.Bacc(trn_type="TRN2", name="attn8v6")

    xt = nc.dram_tensor("xt", [128, NTQ, C // 128, TQ], BF16,
                        kind="ExternalInput")
    wqkt = nc.dram_tensor("wqkt", [C, 2 * CL], BF16, kind="ExternalInput")
    wvt = nc.dram_tensor("wvt", [C, CL], BF16, kind="ExternalInput")
    wot = nc.dram_tensor("wot", [CL, C], BF16, kind="ExternalInput")
    cosb = nc.dram_tensor("cosb", [96, T], BF16, kind="ExternalInput")
    sinb = nc.dram_tensor("sinb", [96, T], BF16, kind="ExternalInput")
    tri2 = nc.dram_tensor("tri2", [128, 2 * 128], BF16, kind="ExternalInput")
    e6 = nc.dram_tensor("e6", [6, NPAIR * 128], BF16, kind="ExternalInput")
    out = nc.dram_tensor("out", [C, T], BF16, kind="ExternalOutput")

    # qk-projection M-tiles (wqkt column order, host-built):
    #   tile 0 [128] : r1 rows [96] = (q h0..h5 | k h0..h5) x dims 0:8
    #                  + r2a rows [32] = tensor-heads 0..3 x dims 8:16
    #   tile 1 [128] : r2b rows [64] = tensor-heads 4..11 x dims 8:16
    #                  + pass rows 0:64
    #   tiles 2..5   : pass rows 64:576
    # pass row order: for blk 0..5 (q pairs then k pairs):
    #   h_even dims 16:64 (48 rows), h_odd dims 16:64 (48 rows)
    A_ORDER = [0, 2, 4, 1, 3, 5, 6, 8, 10, 7, 9, 11]

    BLK_ORDER = [0, 3, 1, 4, 2, 5]

    def pass_dest(row):
        g, r = divmod(row, 96)
        part = 64 * (r // 48) + 16 + (r % 48)
        return BLK_ORDER[g], part

    with tile.TileContext(nc) as tc:
        with (
            tc.tile_pool(name="persist", bufs=1) as pp,
            tc.tile_pool(name="weights", bufs=1) as wp,
            tc.tile_pool(name="pstage", bufs=2) as psg,
            tc.tile_pool(name="ropet", bufs=1) as rtp,
            tc.tile_pool(name="rots", bufs=2) as rop,
            tc.tile_pool(name="expp", bufs=3) as xpp,
            tc.tile_pool(name="ounp", bufs=2) as onp,
            tc.tile_pool(name="osbp", bufs=2) as osp,
            tc.tile_pool(name="misc", bufs=2) as msc,
            tc.tile_pool(name="flex", bufs=2, space="PSUM") as flx,
            tc.tile_pool(name="sps", bufs=2, space="PSUM") as sps,
            tc.tile_pool(name="ops", bufs=1, space="PSUM") as ops,
        ):
            qk_sb = pp.tile([128, 2 * NPAIR, T], BF16, tag="qk")
            v_sb = pp.tile([128, NKT, NHL, HD + 1], BF16, tag="v")
            cos_t = pp.tile([96, T], BF16, tag="cos")
            sin_t = pp.tile([96, T], BF16, tag="sin")
            tri_t = pp.tile([128, 2 * 128], BF16, tag="tri")
            e6_t = pp.tile([6, NPAIR * 128], BF16, tag="e6")
            rs6_t = pp.tile([6, TQ], F32, tag="rs6")
            rf6_t = pp.tile([6, TQ], F32, tag="rf6")
            rinv6_t = pp.tile([6, TQ], BF16, tag="rinv6")
            warm_t = pp.tile([128, TQ], BF16, tag="warm")
            x_all = wp.tile([128, NTQ, C // 128, TQ], BF16, tag="xall")

            wqk_t = wp.tile([128, C // 128, 2 * CL], BF16, tag="wqk")
            wv_t = wp.tile([128, C // 128, CL], BF16, tag="wv")
            wo_t = wp.tile([128, NPAIR, C], BF16, tag="wo")

            wqk_r = wqkt.rearrange("(co p) m -> co p m", p=128)
            out_r = out.rearrange("(do p) t -> p do t", p=128)

            # ---- PE warm-up: keep the HAM activity window busy while the
            # prologue DMAs stream (a cold PE runs at 1.2 instead of 2.4GHz
            # for the first ~3.4us of real work otherwise). ----
            nc.gpsimd.memset(warm_t.bitcast(U16)[:, :], 0x3F80)
            wps = flx.tile([128, TQ], F32, tag="flex")
            for i in range(14):
                nc.tensor.matmul(wps, warm_t[:, 0:128], warm_t,
                                 start=(i == 0), stop=(i == 13))

            # ---- prologue DMAs: x tile 0 + qk weights first (they gate the
            # first rope matmuls), split across the ACT and SP rings ----
            nc.scalar.dma_start(x_all[:, 0], xt[:, 0])
            for c in range(3):
                nc.scalar.dma_start(wqk_t[:, c], wqk_r[c])
            for c in range(3, C // 128):
                nc.sync.dma_start(wqk_t[:, c], wqk_r[c])
            nc.sync.dma_start(cos_t, cosb[:, :])
            nc.sync.dma_start(sin_t, sinb[:, :])
            nc.scalar.dma_start(x_all[:, 1:NTQ], xt[:, 1:NTQ])
            nc.sync.dma_start(
                wv_t, wvt.rearrange("(co p) m -> p co m", p=128))
            nc.sync.dma_start(tri_t, tri2[:, :])
            nc.sync.dma_start(e6_t, e6[:, :])
            # ones-columns of v (the rowsum trick): memset only those 96
            # strided columns, not the whole tile.
            nc.gpsimd.memset(v_sb.bitcast(U16)[:, :, :, HD:HD + 1], 0x3F80)

            def proj_gen(jt):
                """qkv projections for t-tile jt into qk_sb / v_sb.

                Generator, rope-first; yields at chunk boundaries where the
                main loop may interleave attention kt-steps.  Chunk map
                (gates reference these): rope 1-4, pass2 5-6, v0..v3 at
                7-8/9-10/11-12/13-14, pass3 15-16, pass4 17-18, pass5 19-20.
                """
                ts = slice(jt * TQ, (jt + 1) * TQ)
                x_jt = x_all[:, jt]

                def pass_scatter(src_ps, mt):
                    r0 = 0 if mt > 1 else 64          # first pass row in tile
                    prow0 = 128 * mt + r0 - 192       # pass-row index of r0
                    stg = psg.tile([128, TQ], BF16, tag="pstg")
                    nc.vector.tensor_copy(stg[r0:128], src_ps[r0:128])
                    row = prow0
                    while row < prow0 + 128 - r0:
                        blk, part = pass_dest(row)
                        run = min(prow0 + 128 - r0 - row, 48 - row % 48)
                        sr = row - prow0 + r0
                        nc.sync.dma_start(
                            qk_sb[part:part + run, blk, ts],
                            stg[sr:sr + run])
                        row += run

                def gen_pass(mt):
                    ps = flx.tile([128, TQ], F32, tag="flex")
                    for c in range(C // 128):
                        nc.tensor.matmul(
                            ps, wqk_t[:, c, 128 * mt:128 * (mt + 1)],
                            x_jt[:, c], start=(c == 0),
                            stop=(c == C // 128 - 1))
                        if c == 2:
                            yield
                    pass_scatter(ps, mt)
                    yield

                def gen_rope():
                    # psA/psB allocated back-to-back: no yield between them.
                    psA = flx.tile([128, TQ], F32, tag="flex")
                    psB = flx.tile([128, TQ], F32, tag="flex")
                    for mt, psm in ((0, psA), (1, psB)):
                        for c in range(C // 128):
                            nc.tensor.matmul(
                                psm, wqk_t[:, c, 128 * mt:128 * (mt + 1)],
                                x_jt[:, c], start=(c == 0),
                                stop=(c == C // 128 - 1))
                            if c == 2 and mt == 1:
                                yield
                    yield
                    # rot1 = r1*cos - r2*sin ; rot2 = r2*cos + r1*sin
                    # r1 = psA[0:96]; r2 = psB[0:64] ++ psA[96:128]
                    t1 = rtp.tile([96, TQ], F32, tag="t1")
                    t2 = rtp.tile([96, TQ], F32, tag="t2")
                    rot = rop.tile([96, 2, TQ], BF16, tag="rot")
                    rot1 = rot[:, 0, :]
                    rot2 = rot[:, 1, :]
                    nc.vector.tensor_tensor(t1, psA[0:96], cos_t[:, ts], MUL)
                    nc.vector.tensor_tensor(
                        t2[0:64], psB[0:64], sin_t[0:64, ts], MUL)
                    nc.vector.tensor_tensor(
                        t2[64:96], psA[96:128], sin_t[64:96, ts], MUL)
                    nc.vector.tensor_tensor(rot1, t1, t2, SUB)
                    t3 = rtp.tile([96, TQ], F32, tag="t1")
                    t4 = rtp.tile([96, TQ], F32, tag="t2")
                    nc.vector.tensor_tensor(
                        t3[0:64], psB[0:64], cos_t[0:64, ts], MUL)
                    nc.vector.tensor_tensor(
                        t3[64:96], psA[96:128], cos_t[64:96, ts], MUL)
                    nc.vector.tensor_tensor(t4, psA[0:96], sin_t[:, ts], MUL)
                    nc.vector.tensor_tensor(rot2, t3, t4, ADD)
                    # tile 1 rows 64:128 are pass rows
                    pass_scatter(psB, 1)
                    yield
                    # rope scatter: one DMA per tensor-head, pair-0 heads
                    # first; alternate SP/gpsimd rings (issue cost ~600ns
                    # each; either ring alone would serialize the window).
                    for n, a in enumerate((0, 1, 6, 7, 2, 3, 8, 9, 4, 5,
                                           10, 11)):
                        i = A_ORDER.index(a)
                        tn, hl = divmod(a, NHL)
                        blk = (0 if tn == 0 else NPAIR) + hl // 2
                        base = 64 * (hl % 2)
                        eng = nc.sync if n % 2 == 0 else nc.gpsimd
                        eng.dma_start(qk_sb[base:base + 16, blk, ts],
                                      rot[8 * i:8 * i + 8, :, :])
                    yield

                def gen_v(vt):
                    pvf = flx.tile([128, TQ], F32, tag="flex")
                    pv = pvf[:, 0:CL]
                    kt0 = jt * (TQ // 128) + vt
                    for c in range(C // 128):
                        nc.tensor.matmul(
                            pv, x_jt[:, c, vt * 128:(vt + 1) * 128],
                            wv_t[:, c], start=(c == 0),
                            stop=(c == C // 128 - 1))
                        if c == 2:
                            yield
                    nc.vector.tensor_copy(
                        v_sb[:, kt0, :, 0:HD],
                        pv.rearrange("p (h d) -> p h d", d=HD))
                    yield

                parts = ([gen_rope(), gen_pass(2)] +
                         [gen_v(vt) for vt in range(TQ // 128)] +
                         [gen_pass(3), gen_pass(4), gen_pass(5)])
                for g in parts:
                    yield from g
                if jt == 0:
                    nc.scalar.dma_start(
                        wo_t, wot.rearrange("(po p) m -> p po m", p=128))

            class Task:
                def __init__(self, gen, total):
                    self.gen = gen
                    self.total = total
                    self.count = 0
                    self.done = False

                def step(self):
                    if self.done:
                        return False
                    try:
                        next(self.gen)
                        self.count += 1
                        return True
                    except StopIteration:
                        self.done = True
                        return False

                def remaining(self):
                    return 0 if self.done else self.total - self.count

            pending = []     # FIFO of Tasks across windows

            def pump():
                while pending:
                    if pending[0].step():
                        return True
                    pending.pop(0)
                return False

            def norm_pair(p, oun_all, o_sb):
                """bc broadcast (PE) + normalize (DVE) for pair p."""
                bc = flx.tile([128, TQ], F32, tag="flex")
                nc.tensor.matmul(bc, e6_t[:, p * 128:(p + 1) * 128],
                                 rinv6_t, start=True, stop=True)
                nc.vector.tensor_tensor(
                    o_sb[0:64, p], oun_all[0:64, p, 0], bc[0:64], MUL)
                nc.vector.tensor_tensor(
                    o_sb[64:128, p], oun_all[0:64, p, 1], bc[64:128], MUL)

            def outproj_gen(jq, oun_all):
                """normalize + output projection for q-tile jq (pumped
                inside the next window)."""
                qs = slice(jq * TQ, (jq + 1) * TQ)
                o_sb = osp.tile([128, NPAIR, TQ], BF16, tag="osb")
                for p in range(NPAIR):
                    norm_pair(p, oun_all, o_sb)
                    yield
                ost = msc.tile([128, C // 128, TQ], BF16, tag="ost")
                for dt in range(C // 128):
                    po = flx.tile([128, TQ], F32, tag="flex")
                    for p in range(NPAIR):
                        nc.tensor.matmul(
                            po, wo_t[:, p, dt * 128:(dt + 1) * 128],
                            o_sb[:, p], start=(p == 0), stop=(p == NPAIR - 1))
                    nc.vector.tensor_copy(ost[:, dt], po)
                    if dt == 2:
                        nc.gpsimd.dma_start(out_r[:, 0:3, qs], ost[:, 0:3])
                    yield
                nc.gpsimd.dma_start(out_r[:, 3:6, qs], ost[:, 3:6])

            def attn(jq, proj_task, n_chunks, tail_sb=None):
                """causal attention for q-tile jq; pumps pending chunks
                between kt-steps.  proj_task is proj(jq)'s Task: its chunk
                counter gates emission (sc of pair p needs its q/k blocks
                scattered; av of a new k-tile needs its v chunk) -- an
                attention instruction emitted before its producer chunks
                would head-of-line deadlock the in-order PE queue."""
                qs = slice(jq * TQ, (jq + 1) * TQ)
                oun_all = onp.tile([128, NPAIR, 2, TQ], F32, tag="oun")
                nkt = 4 * (jq + 1)
                per_step = n_chunks / (NPAIR * nkt)
                acc = 0.0
                PAIR_GATES = [6, 18, 20]

                def pump_proj(k):
                    while proj_task.count < k and not proj_task.done:
                        proj_task.step()

                for p in range(NPAIR):
                    pump_proj(PAIR_GATES[p])
                    qb = qk_sb[:, p, qs]
                    kb = qk_sb[:, NPAIR + p, :]
                    o_ps = ops.tile([128, 2, TQ], F32, tag="o")
                    pend = []

                    def issue_av(kt, ep, a):
                        m = kt - 4 * jq
                        if m >= 0:
                            pump_proj(8 + 2 * m)
                        for h in range(2):
                            nc.tensor.matmul(
                                o_ps[0:HD + 1, h, a:TQ],
                                v_sb[:, kt, 2 * p + h, :], ep[:, h, a:TQ],
                                start=(kt == 0), stop=(kt == nkt - 1))

                    for kt in range(nkt):
                        m = kt - 4 * jq
                        a = 0 if m < 0 else 128 * m
                        ks = slice(kt * 128, (kt + 1) * 128)
                        sg = sps.tile([128, 2, TQ], F32, tag="s")
                        nc.tensor.matmul(
                            sg[:, 0, a:TQ], kb[0:64, ks], qb[0:64, a:TQ],
                            start=True, stop=True, tile_position=(0, 0))
                        nc.tensor.matmul(
                            sg[:, 1, a:TQ], kb[64:128, ks], qb[64:128, a:TQ],
                            start=True, stop=True, tile_position=(64, 0))
                        ep = xpp.tile([128, 2, TQ], BF16, tag="e")
                        nc.scalar.activation(ep[:, :, a:TQ], sg[:, :, a:TQ],
                                             AF.Exp, scale=0.125)
                        if m >= 0:
                            nc.gpsimd.tensor_tensor(
                                ep[:, :, a:a + 128], ep[:, :, a:a + 128],
                                tri_t.rearrange("p (h q) -> p h q", h=2), MUL)
                        pend.append((kt, ep, a))
                        if len(pend) > 2:
                            issue_av(*pend.pop(0))
                        acc += per_step
                        while acc >= 1.0:
                            pump()
                            acc -= 1.0
                    for job in pend:
                        issue_av(*job)
                    # evict o accumulator (+ rowsums in row HD) on DVE: ACT
                    # is the attention pacer, DVE has slack.
                    nc.vector.tensor_copy(oun_all[0:HD + 1, p],
                                          o_ps[0:HD + 1])
                    if tail_sb is not None:
                        # per-pair rowsum -> 1/x -> bc/normalize, so the
                        # final output projection mostly overlaps the
                        # remaining pairs' attention.  Drain pending chunks
                        # first: outproj(jq-1)'s bc matmuls must be emitted
                        # before rinv6 rows are overwritten with this
                        # window's values.
                        if p == 0:
                            while pump():
                                pass
                        # DVE partition starts must be 32-aligned, so the
                        # 1/x runs on all 6 rows each time (cost is set by
                        # free size, not partitions); rows of later pairs
                        # hold stale-but-finite values that norm_pair's
                        # one-hot bc stationary multiplies by zero.
                        nc.gpsimd.dma_start(rs6_t[2 * p:2 * p + 2],
                                            oun_all[HD:HD + 1, p])
                        nc.vector.reciprocal_approx_fast(
                            rf6_t[:, :], rs6_t[:, :])
                        nc.vector.tensor_copy(rinv6_t, rf6_t)
                        norm_pair(p, oun_all, tail_sb)
                # batched rowsum gather + fast reciprocal (51 ULP; the
                # correctness gate is 2e-2) for the non-tail windows.
                if tail_sb is None:
                    nc.gpsimd.dma_start(rs6_t, oun_all[HD:HD + 1, :, :, :])
                    nc.vector.reciprocal_approx_fast(rf6_t[:, :], rs6_t[:, :])
                    nc.vector.tensor_copy(rinv6_t, rf6_t)
                return oun_all

            def tail_outproj(jq, o_sb):
                """output projection for the last q-tile: dt0/dt1's pair-0/1
                matmuls are emitted first so they execute during the pair-2
                normalize chain; PSUM evictions alternate DVE/ACT (ACT is
                idle after the last exp)."""
                qs = slice(jq * TQ, (jq + 1) * TQ)
                ost = msc.tile([128, C // 128, TQ], BF16, tag="ost")
                po0 = flx.tile([128, TQ], F32, tag="flex")
                po1 = flx.tile([128, TQ], F32, tag="flex")
                for dt, po in ((0, po0), (1, po1)):
                    for p in range(2):
                        nc.tensor.matmul(
                            po, wo_t[:, p, dt * 128:(dt + 1) * 128],
                            o_sb[:, p], start=(p == 0), stop=False)
                for dt, po in ((0, po0), (1, po1)):
                    nc.tensor.matmul(
                        po, wo_t[:, 2, dt * 128:(dt + 1) * 128],
                        o_sb[:, 2], start=False, stop=True)
                    if dt == 0:
                        nc.scalar.copy(ost[:, 0], po)
                    else:
                        nc.vector.tensor_copy(ost[:, 1], po)
                for dt in range(2, C // 128):
                    po = flx.tile([128, TQ], F32, tag="flex")
                    for p in range(NPAIR):
                        nc.tensor.matmul(
                            po, wo_t[:, p, dt * 128:(dt + 1) * 128],
                            o_sb[:, p], start=(p == 0), stop=(p == NPAIR - 1))
                    if dt % 2 == 0:
                        nc.scalar.copy(ost[:, dt], po)
                    else:
                        nc.vector.tensor_copy(ost[:, dt], po)
                    if dt == 3:
                        nc.gpsimd.dma_start(out_r[:, 0:3, qs], ost[:, 0:3])
                nc.gpsimd.dma_start(out_r[:, 3:6, qs], ost[:, 3:6])

            # ---- main pipeline ----
            tasks = {0: Task(proj_gen(0), 20)}
            pending.append(tasks[0])
            ouns = [None] * NTQ
            tail_sb = None
            for jq in range(NTQ):
                if jq >= 1:
                    pending.append(Task(outproj_gen(jq - 1, ouns[jq - 1]), 9))
                if jq + 1 < NTQ:
                    tasks[jq + 1] = Task(proj_gen(jq + 1), 20)
                    pending.append(tasks[jq + 1])
                else:
                    tail_sb = osp.tile([128, NPAIR, TQ], BF16, tag="osb")
                spill = 6 if jq + 1 < NTQ else 0
                n_chunks = max(0, sum(t.remaining() for t in pending) - spill)
                ouns[jq] = attn(jq, tasks[jq], n_chunks,
                                tail_sb=(tail_sb if jq == NTQ - 1 else None))
            while pump():
                pass
            tail_outproj(NTQ - 1, tail_sb)

    nc.compile()
    return nc


def _host_inputs(x, w_qkv, w_out):
    """Build per-core input dicts. Core i: batch i//2, head-group i%2."""
    import ml_dtypes

    BF = ml_dtypes.bfloat16
    xf = np.asarray(x, dtype=np.float32)
    w3 = np.asarray(w_qkv, dtype=np.float32).reshape(3, NH, HD, C)
    wo = np.asarray(w_out, dtype=np.float32)

    per_group = []
    for g in range(2):
        hs = range(g * NHL, (g + 1) * NHL)
        A_ORDER = [0, 2, 4, 1, 3, 5, 6, 8, 10, 7, 9, 11]
        rows = []
        # M-tile 0: r1 (A_ORDER tensor-heads x dims 0:8) + r2a (last 4 of
        # A_ORDER x dims 8:16); M-tile 1 rows 0:64: r2b (first 8 x 8:16)
        for a in A_ORDER:
            tn, hl = divmod(a, NHL)
            rows.append(w3[tn, g * NHL + hl, 0:8])
        for a in A_ORDER[8:12]:
            tn, hl = divmod(a, NHL)
            rows.append(w3[tn, g * NHL + hl, 8:16])
        for a in A_ORDER[0:8]:
            tn, hl = divmod(a, NHL)
            rows.append(w3[tn, g * NHL + hl, 8:16])
        # pass rows: blocks in BLK_ORDER; per blk h_even, h_odd
        for blk in (0, 3, 1, 4, 2, 5):
            tn, pr = divmod(blk, NPAIR)
            for ho in range(2):
                rows.append(w3[tn, g * NHL + 2 * pr + ho, 16:64])
        wqk = np.concatenate(rows, axis=0)                  # [768, C]
        wqkt = np.ascontiguousarray(wqk.T).astype(BF)       # [C, 768]
        wv = w3[2, list(hs)].reshape(CL, C)                 # [384, C]
        wvt = np.ascontiguousarray(wv.T).astype(BF)
        wotr = np.ascontiguousarray(
            wo[:, g * CL:(g + 1) * CL].T).astype(BF)        # [384, 768]
        per_group.append((wqkt, wvt, wotr))

    j = np.arange(RD // 2, dtype=np.float64)
    freqs = 1.0 / (10000.0 ** (2 * j / RD))
    t = np.arange(T, dtype=np.float64)
    ang = t[None, :] * freqs[:, None]                        # [8, T]
    cosb = np.ascontiguousarray(np.tile(np.cos(ang), (12, 1))).astype(BF)
    sinb = np.ascontiguousarray(np.tile(np.sin(ang), (12, 1))).astype(BF)

    kk = np.arange(128)[:, None]
    qq = np.arange(128)[None, :]
    tri = (kk <= qq).astype(BF)
    tri2 = np.ascontiguousarray(np.concatenate([tri, tri], axis=1))
    e6 = np.zeros((6, NPAIR * 128), dtype=BF)
    for p in range(NPAIR):
        e6[2 * p, p * 128:p * 128 + 64] = 1.0
        e6[2 * p + 1, p * 128 + 64:(p + 1) * 128] = 1.0

    in_maps = []
    for i in range(8):
        b, g = divmod(i, 2)
        wqkt, wvt, wotr = per_group[g]
        # x packed [p, jt, c, tq] so each x-tile DMA is fully contiguous
        xb = np.ascontiguousarray(xf[b].T).astype(BF)        # [C, T]
        xp = np.ascontiguousarray(
            xb.reshape(C // 128, 128, NTQ, TQ).transpose(1, 2, 0, 3))
        in_maps.append({
            "xt": xp,
            "wqkt": wqkt, "wvt": wvt, "wot": wotr,
            "cosb": cosb, "sinb": sinb, "tri2": tri2, "e6": e6,
        })
    return in_maps


def kernel(x, w_qkv, w_out, _trace=False):
    from concourse.bass_utils import run_bass_kernel_spmd

    if "nc" not in _cache:
        _cache["nc"] = _build()
    nc = _cache["nc"]
    in_maps = _host_inputs(x, w_qkv, w_out)
    res = run_bass_kernel_spmd(nc, in_maps, core_ids=list(range(8)),
                               trace=_trace)
    _cache["last_result"] = res
    out = np.empty((B, T, C), dtype=np.float32)
    for b in range(B):
        acc = res.results[2 * b]["out"].astype(np.float32) + \
            res.results[2 * b + 1]["out"].astype(np.float32)
        out[b] = acc.T
    return out
